# revision 10
# baseline (speedup 1.0000x reference)
"""ChebyConv (K=3) GNN kernel for 8 Trainium2 NeuronCores.

out = x@(W0-W2) + L@c + bias,  c = x@W1 + (L@x)@(2*W2)

v2 design (vs v1 masked-matmul with 512-wide dest quads):
- Dest rows split across 8 cores; per core, dests grouped in BLOCKS of 128.
- Edges of a block chunked 128-at-a-time; DVE/GpSimd build a [128 edge,
  128 dest] fp16 one-hot*val mask; PE accumulates psum[64 feat, 128 dest]
  per block (4x less mask work per edge than v1's 512-wide masks).
- Gather tables stored fp16 padded to 256B rows (gather granularity), so
  gathered data is directly usable as fp16 matmul lhsT - no convert pass.
- Gather calls batched per (super-block of 4 blocks, source window).
- Mask building split between Vector (DVE) and GpSimd engines.
- Hop-1 c rows AllGathered in 7 batches (14 blocks each), pipelined.
"""

import os
import numpy as np

CHUNK = 128          # edges per mask-matmul chunk (PE contraction dim)
DB = 128             # dest rows per block (mask free dim / psum region)
SB = 2               # blocks per super-block (gather-call batching)
W = 25088            # source-window rows per gather window (int16 idx limit)
NB_AG = 14           # blocks per AllGather batch
NC = 8
MAX_CALL_CHUNKS = 8   # chunks per dma_gather call (spread queues)
POOL_MASK_MOD = 10   # chunk j built on gpsimd if j % MOD < POOL_MASK_CNT
POOL_MASK_CNT = 3

LAST_EXEC_NS = None


def _edge_layout(win_of_edge, blk_of_edge, r, c, v, idx_of_edge, nblk):
    """Shared static slot layout for one spmm.

    Chunk space ordered by (sblock, window, block-within-sblock) so each
    (sblock, window) is a contiguous chunk range -> one gather call.
    """
    nsb = -(-nblk // SB)
    ngrp = nsb * 4 * SB
    counts = np.zeros((NC, ngrp), dtype=np.int64)
    keys = []
    orders = []
    for ci in range(NC):
        b = blk_of_edge[ci]
        key = (b // SB) * (4 * SB) + win_of_edge[ci] * SB + (b % SB)
        order = np.lexsort((c[ci], key))
        keys.append(key[order])
        orders.append(order)
        counts[ci] = np.bincount(key, minlength=ngrp)
    cg = np.maximum(1, -(-counts.max(axis=0) // CHUNK))
    # blocks beyond nblk (in a partial last sblock) get no chunks
    for g in range(ngrp):
        blk = (g // (4 * SB)) * SB + (g % SB)
        if blk >= nblk:
            cg[g] = 0
    grp_chunk_off = np.concatenate(([0], np.cumsum(cg)))
    tot_chunks = int(grp_chunk_off[-1])
    tot_slots = tot_chunks * CHUNK

    # per (sblock, window): contiguous chunk range -> gather calls
    calls = []   # (sblock, window, chunk0, nchunks_call, rel_chunk_in_tile)
    sw_tiles = []  # (sblock, window, chunk0, nchunks) per gather tile
    max_sw_chunks = 0
    max_sb_chunks = 0
    for s in range(nsb):
        s_chunks = 0
        for w in range(4):
            g0 = s * (4 * SB) + w * SB
            c0, c1 = int(grp_chunk_off[g0]), int(grp_chunk_off[g0 + SB])
            sw_tiles.append((s, w, c0, c1 - c0))
            max_sw_chunks = max(max_sw_chunks, c1 - c0)
            s_chunks += c1 - c0
            k = c0
            while k < c1:
                n = min(MAX_CALL_CHUNKS, c1 - k)
                calls.append((s, w, k, n, k - c0))
                k += n
        max_sb_chunks = max(max_sb_chunks, s_chunks)

    # per block: list of (chunk_index, sw_tile_index, rel_chunk) in chunk order
    blk_chunks = []
    for blk in range(nblk):
        s, bi = blk // SB, blk % SB
        lst = []
        for w in range(4):
            g = s * (4 * SB) + w * SB + bi
            for k in range(int(grp_chunk_off[g]), int(grp_chunk_off[g + 1])):
                lst.append((k, s * 4 + w, k - int(grp_chunk_off[s * (4 * SB) + w * SB])))
        blk_chunks.append(lst)

    per_core = []
    for ci in range(NC):
        order = orders[ci]
        key = keys[ci]
        cnt = counts[ci]
        rr = np.zeros(tot_slots, dtype=np.float32)
        vv = np.zeros(tot_slots, dtype=np.float32)
        ii = np.zeros(tot_slots, dtype=np.int16)
        within = np.arange(len(key)) - np.repeat(
            np.concatenate(([0], np.cumsum(cnt)))[:-1], cnt)
        slot = grp_chunk_off[key] * CHUNK + within
        rr[slot] = (r[ci][order] & (DB - 1)).astype(np.float32)
        vv[slot] = v[ci][order].astype(np.float32)
        ii[slot] = idx_of_edge[ci][order].astype(np.int16)
        rr_t = np.ascontiguousarray(rr.reshape(tot_chunks, CHUNK).T)
        vv_t = np.ascontiguousarray(vv.reshape(tot_chunks, CHUNK).T)
        iw = np.ascontiguousarray(ii.reshape(tot_slots // 16, 16).T)
        iw = np.tile(iw, (8, 1))
        per_core.append((rr_t, vv_t, iw))
    return per_core, dict(tot_chunks=tot_chunks, tot_slots=tot_slots,
                          calls=calls, sw_tiles=sw_tiles,
                          max_sw_chunks=max_sw_chunks,
                          max_sb_chunks=max_sb_chunks,
                          blk_chunks=blk_chunks, nsb=nsb)


def _host_prep(x, rows, cols, vals, weight, bias):
    N, F = x.shape
    assert F == 64
    assert N % NC == 0
    shard = N // NC
    nblk = -(-shard // DB)
    vrows = nblk * DB
    assert 4 * W >= N
    tbl2_rows = NC * vrows
    assert tbl2_rows % 4 == 0 and tbl2_rows // 4 <= W
    assert vrows % NB_AG == 0 or True
    nag = -(-nblk // NB_AG)
    ag_rows = NB_AG * DB           # rows per core per AG batch

    rows = np.asarray(rows).astype(np.int64)
    cols = np.asarray(cols).astype(np.int64)
    vals = np.asarray(vals, dtype=np.float32)
    x = np.asarray(x, dtype=np.float32)
    weight = np.asarray(weight, dtype=np.float32)
    bias = np.asarray(bias, dtype=np.float32)

    bounds = np.searchsorted(rows, np.arange(NC + 1) * shard)
    r_, c_, v_ = [], [], []
    for ci in range(NC):
        e0, e1 = bounds[ci], bounds[ci + 1]
        r_.append(rows[e0:e1] - ci * shard)
        c_.append(cols[e0:e1])
        v_.append(vals[e0:e1])

    # spmm1: gather from x2 (fp16 padded rows); window = col // W
    q1 = [c // W for c in c_]
    i1 = [c - q * W for c, q in zip(c_, q1)]
    # spmm2: gather from c_tbl; table row of node j (core r, local lr):
    #   batch = lr // ag_rows; row = batch*(NC*ag_rows) + r*ag_rows + lr%ag_rows
    tix = []
    for c in c_:
        rr = c // shard
        lr = c - rr * shard
        tix.append((lr // ag_rows) * (NC * ag_rows) + rr * ag_rows
                   + (lr % ag_rows))
    q2 = [t // W for t in tix]
    i2 = [t - q * W for t, q in zip(tix, q2)]
    blk_dest = [r // DB for r in r_]

    lay1_cores, lay1 = _edge_layout(q1, blk_dest, r_, c_, v_, i1, nblk)
    lay2_cores, lay2 = _edge_layout(q2, blk_dest, r_, c_, v_, i2, nblk)

    x2 = np.zeros((4 * W, 2 * F), dtype=np.float16)
    x2[:N, :F] = x.astype(np.float16)
    iota = np.tile(np.arange(DB, dtype=np.float16), (128, 1))
    w1 = np.ascontiguousarray(weight[1].astype(np.float16))
    w2s = np.ascontiguousarray((2.0 * weight[2]).astype(np.float16))
    # w0m2b: [65, 64] = (W0 - W2) with bias appended as last contraction row
    w0m2b = np.zeros((F + 1, F), dtype=np.float16)
    w0m2b[:F] = (weight[0] - weight[2]).astype(np.float16)
    w0m2b[F] = bias.astype(np.float16)

    core_inputs = []
    for ci in range(NC):
        rr1, vv1, iw1 = lay1_cores[ci]
        rr2, vv2, iw2 = lay2_cores[ci]
        # xq: [65, vrows] fp16; row 64 = ones (bias trick)
        xq = np.zeros((F + 1, vrows), dtype=np.float16)
        lo = ci * shard
        hi = min(lo + vrows, N)
        xq[:F, :hi - lo] = x[lo:hi].T.astype(np.float16)
        xq[F, :] = 1.0
        core_inputs.append({
            "xg": x2, "xq": xq,
            "rr1": rr1, "vv1": vv1, "i1": iw1,
            "rr2": rr2, "vv2": vv2, "i2": iw2,
            "iota": iota, "w1": w1, "w2s": w2s, "w0m2b": w0m2b,
        })

    meta = dict(N=N, F=F, shard=shard, nblk=nblk, vrows=vrows,
                nag=nag, ag_rows=ag_rows, lay1=lay1, lay2=lay2)
    return core_inputs, meta


def _build_program(meta):
    import concourse.bass as bass  # noqa
    import concourse.mybir as mybir
    import concourse.tile as tile
    from concourse import bacc

    F = meta["F"]
    nblk = meta["nblk"]
    vrows = meta["vrows"]
    nag, ag_rows = meta["nag"], meta["ag_rows"]
    lay1, lay2 = meta["lay1"], meta["lay2"]
    f32, f16, i16 = mybir.dt.float32, mybir.dt.float16, mybir.dt.int16
    AOP = mybir.AluOpType
    ACTF = mybir.ActivationFunctionType

    nc = bacc.Bacc("TRN2", target_bir_lowering=False, debug=False,
                   num_devices=NC, num_swdge_queues=4)
    xg = nc.dram_tensor("xg", [4 * W, 2 * F], f16, kind="ExternalInput")
    xq = nc.dram_tensor("xq", [F + 1, vrows], f16, kind="ExternalInput")
    edge_dram = {}
    for nm, lay in (("1", lay1), ("2", lay2)):
        edge_dram["rr" + nm] = nc.dram_tensor(
            "rr" + nm, [128, lay["tot_chunks"]], f32, kind="ExternalInput")
        edge_dram["vv" + nm] = nc.dram_tensor(
            "vv" + nm, [128, lay["tot_chunks"]], f32, kind="ExternalInput")
        edge_dram["i" + nm] = nc.dram_tensor(
            "i" + nm, [128, lay["tot_slots"] // 16], i16, kind="ExternalInput")
    iota = nc.dram_tensor("iota", [128, DB], f16, kind="ExternalInput")
    w1 = nc.dram_tensor("w1", [F, F], f16, kind="ExternalInput")
    w2s = nc.dram_tensor("w2s", [F, F], f16, kind="ExternalInput")
    w0m2b = nc.dram_tensor("w0m2b", [F + 1, F], f16, kind="ExternalInput")
    outT = nc.dram_tensor("outT", [F, vrows], f32, kind="ExternalOutput")
    # hop-1 output c, fp16 rows padded to 256B for gather granularity
    c_shard = nc.dram_tensor("c_shard", [vrows, 2 * F], f16)
    c_tbl = nc.dram_tensor("c_tbl", [NC * vrows, 2 * F], f16,
                           addr_space="Shared")

    gq = [0]
    mask_ctr = [0]

    with tile.TileContext(nc) as tc:
        with tc.tile_pool(name="const", bufs=1) as constp, \
             tc.tile_pool(name="edges", bufs=15) as edgep, \
             tc.tile_pool(name="gbuf", bufs=20) as gp, \
             tc.tile_pool(name="mask", bufs=48) as mp, \
             tc.tile_pool(name="acc", bufs=6) as accp, \
             tc.tile_pool(name="ps1", bufs=3, space="PSUM") as ps1, \
             tc.tile_pool(name="ps2", bufs=2, space="PSUM") as ps2:

            iota_t = constp.tile([128, DB], f16)
            nc.sync.dma_start(out=iota_t[:], in_=iota[:])
            w1_t = constp.tile([F, F], f16, tag="w1")
            nc.sync.dma_start(out=w1_t[:], in_=w1[:])
            w2s_t = constp.tile([F, F], f16, tag="w2s")
            nc.sync.dma_start(out=w2s_t[:], in_=w2s[:])
            w0m2b_t = constp.tile([F + 1, F], f16, tag="w0m2b")
            nc.sync.dma_start(out=w0m2b_t[:], in_=w0m2b[:])
            xq_t = constp.tile([F + 1, vrows], f16, tag="xq")
            nc.sync.dma_start(out=xq_t[:], in_=xq[:])

            def emit_ag(b):
                nc.gpsimd.collective_compute(
                    "AllGather", mybir.AluOpType.bypass,
                    replica_groups=[list(range(NC))],
                    ins=[c_shard[b * ag_rows:(b + 1) * ag_rows, :]],
                    outs=[c_tbl[b * NC * ag_rows:(b + 1) * NC * ag_rows, :]])

            def issue_sblock(s, tbl, lay, nm, state):
                """DMA edge tiles + gathers for super-block s of one spmm."""
                maxc = lay["max_sw_chunks"]
                maxsb = lay["max_sb_chunks"]
                sw = [t for t in lay["sw_tiles"] if t[0] == s]
                c0 = sw[0][2]
                c1 = sw[-1][2] + sw[-1][3]
                nch = c1 - c0
                rr_t = edgep.tile([128, maxsb], f32, tag="rr")
                nc.sync.dma_start(out=rr_t[:, :nch],
                                  in_=edge_dram["rr" + nm][:, c0:c1])
                vv_t = edgep.tile([128, maxsb], f32, tag="vv")
                nc.sync.dma_start(out=vv_t[:, :nch],
                                  in_=edge_dram["vv" + nm][:, c0:c1])
                ix_t = edgep.tile([128, maxsb * 8], i16, tag="ix")
                nc.sync.dma_start(out=ix_t[:, :nch * 8],
                                  in_=edge_dram["i" + nm][:, c0 * 8:c1 * 8])
                gt = {}
                for (ss, wq, cw0, ncw) in sw:
                    g16 = gp.tile([128, maxc * 2 * F], f16, tag="g")
                    gt[wq] = (g16, cw0)
                for (ss, wq, k0, ncall, rel) in lay["calls"]:
                    if ss != s:
                        continue
                    g16, cw0 = gt[wq]
                    nidx = ncall * CHUNK
                    nc.gpsimd.dma_gather(
                        out_ap=g16[:, rel * 2 * F:(rel + ncall) * 2 * F]
                            .rearrange("p (c e) -> p c e", e=2 * F),
                        in_ap=tbl[wq * W:(wq + 1) * W, :],
                        idxs_ap=ix_t[:, (k0 - c0) * 8:(k0 - c0) * 8 + nidx // 16],
                        num_idxs=nidx, num_idxs_reg=nidx, elem_size=2 * F,
                        single_packet=False, queue_num=gq[0] % 4)
                    gq[0] += 1
                state[s] = (rr_t, vv_t, gt, c0)

            def spmm_sblock(s, lay, second, mask_engines, state):
                """Masks + matmuls + GEMMs for super-block s of one spmm."""
                rr_t, vv_t, gt, c0 = state.pop(s)
                psum = ps1.tile([F, SB * DB], f32)
                for bi in range(SB):
                    blk = s * SB + bi
                    if blk >= nblk:
                        break
                    chunks = lay["blk_chunks"][blk]
                    pslice = psum[:, bi * DB:(bi + 1) * DB]
                    for jj, (k, swi, relc) in enumerate(chunks):
                        wq = swi % 4
                        g16, cw0 = gt[wq]
                        mask = mp.tile([128, DB], f16)
                        mask_ctr[0] += 1
                        eng = mask_engines[mask_ctr[0] % len(mask_engines)]
                        eng.tensor_scalar(
                            out=mask[:], in0=iota_t[:],
                            scalar1=rr_t[:, k - c0:k - c0 + 1],
                            scalar2=vv_t[:, k - c0:k - c0 + 1],
                            op0=AOP.is_equal, op1=AOP.mult)
                        nc.tensor.matmul(
                            out=pslice,
                            lhsT=g16[:, relc * 2 * F:relc * 2 * F + F],
                            rhs=mask[:],
                            start=(jj == 0),
                            stop=(jj == len(chunks) - 1) and not second)
                    if not second:
                        # c = x@W1 + T1@(2W2) for this block
                        t1t = accp.tile([F, DB], f16, tag="t1t")
                        nc.scalar.activation(out=t1t[:], in_=pslice,
                                             func=ACTF.Copy)
                        psc = ps2.tile([128, F], f32)
                        nc.tensor.matmul(out=psc[:], lhsT=t1t[:], rhs=w2s_t[:],
                                         start=True, stop=False)
                        nc.tensor.matmul(out=psc[:],
                                         lhsT=xq_t[:F, blk * DB:(blk + 1) * DB],
                                         rhs=w1_t[:], start=False, stop=True)
                        c_sb = accp.tile([128, F], f16, tag="csb")
                        nc.scalar.activation(out=c_sb[:], in_=psc[:],
                                             func=ACTF.Copy)
                        nc.sync.dma_start(
                            out=c_shard[blk * DB:(blk + 1) * DB, :F],
                            in_=c_sb[:])
                    else:
                        # out^T = psum + (W0-W2)^T x^T + bias (ones-row trick)
                        nc.tensor.matmul(
                            out=pslice, lhsT=w0m2b_t[:],
                            rhs=xq_t[:, blk * DB:(blk + 1) * DB],
                            start=False, stop=True)
                        o_sb = accp.tile([F, DB], f32, tag="osb")
                        nc.scalar.activation(out=o_sb[:], in_=pslice,
                                             func=ACTF.Copy)
                        nc.sync.dma_start(
                            out=outT[:, blk * DB:(blk + 1) * DB],
                            in_=o_sb[:])

            # masks on DVE only: gpsimd tensor_scalar measured 2.2us/op on HW
            # and head-of-line blocks the gather issues on the Pool sequencer
            mask_engines = [nc.vector]

            PREFETCH = 3
            nsb1 = lay1["nsb"]
            ag_done = 0
            state1 = {}
            for s in range(nsb1 + PREFETCH):
                if s < nsb1:
                    issue_sblock(s, xg, lay1, "1", state1)
                if s >= PREFETCH:
                    spmm_sblock(s - PREFETCH, lay1, False, mask_engines,
                                state1)
                    ready = min(nag, max(0, ((s - PREFETCH) * SB) // NB_AG))
                    while ag_done < ready:
                        emit_ag(ag_done)
                        ag_done += 1
            while ag_done < nag:
                emit_ag(ag_done)
                ag_done += 1
            # hop2 gathers read c_tbl written by the AllGathers; make the
            # ordering explicit rather than relying on timing
            tc.strict_bb_all_engine_barrier()
            nsb2 = lay2["nsb"]
            state2 = {}
            for s in range(nsb2 + PREFETCH):
                if s < nsb2:
                    issue_sblock(s, c_tbl, lay2, "2", state2)
                if s >= PREFETCH:
                    spmm_sblock(s - PREFETCH, lay2, True, mask_engines,
                                state2)

    nc.compile()
    return nc


def kernel(**inputs):
    global LAST_EXEC_NS
    core_inputs, meta = _host_prep(
        inputs["x"], inputs["rows"], inputs["cols"], inputs["vals"],
        inputs["weight"], inputs["bias"])
    nc = _build_program(meta)

    trace = os.environ.get("KERNEL_TRACE", "0") == "1"
    if trace:
        try:
            import sys, types  # noqa
            if "antenv.axon_hooks" not in sys.modules:
                import antenv
                from trn_agent_boot.trn_boot import _ntff_profile_via_ctypes
                mod = types.ModuleType("antenv.axon_hooks")
                hook = _ntff_profile_via_ctypes("/opt/axon/libaxon_pjrt.so")
                mod.get_axon_ntff_profile_hook = lambda: hook
                sys.modules["antenv.axon_hooks"] = mod
                antenv.axon_hooks = mod
        except Exception:
            trace = False

    from concourse.bass_utils import run_bass_kernel_spmd
    res = run_bass_kernel_spmd(nc, core_inputs, list(range(NC)), trace=trace)
    LAST_EXEC_NS = res.exec_time_ns

    N, F, shard = meta["N"], meta["F"], meta["shard"]
    out = np.empty((N, F), dtype=np.float32)
    for ci in range(NC):
        out[ci * shard:(ci + 1) * shard] = res.results[ci]["outT"][:, :shard].T
    return out
